# revision 12
# baseline (speedup 1.0000x reference)
"""Trainium2 Bass kernel for CrossAttention (B=8, Nq=4096, Nk=77, H=16, D=64).

Sharding: data-parallel over batch — one batch element per NeuronCore (8 cores).

End-to-end latency strategy (the dominant cost is host<->device transfer over
the PJRT tunnel plus compile, not kernel exec):
  - All big DRAM I/O is bf16: x upload 64 MB, out download 64 MB.
  - Weights are shipped ONCE (replicated PartitionSpec()) instead of 8 copies.
  - Uploads stream on a worker thread while the main thread builds the Bass
    module and jit-compiles the sharded executable.
  - Output is fetched per-shard in parallel threads, then upcast to f32.

Per-core dataflow (PE matmuls in f32r; bf16 only at the DMA boundary):
  - transpose x chunk on PE (identity matmul)         xT   [1024, CH]
  - qT = Wq^T-free matmul: lhsT=Wq[k,m], rhs=xT[k]    qT   [1024, CH]
  - kT = lhsT=Wk slice, rhs=cT (context transposed)   kT   [1024, 77]
  - v  = lhsT=cT, rhs=Wv (natural layout)             v    [77, 1024] (+ ones col per head)
  - simT_h = lhsT=kT_h [64,77], rhs=qT_h [64,CH]      simT [77, CH]
  - expT_h = exp(scale*simT) on ACT                   expT [77, CH]
  - avT_h  = lhsT=v_aug_h [77,65], rhs=expT           avT  [65, CH] (row 64 = softmax denom)
  - recip + broadcast via tiny matmul, DVE multiply   outT [1024, CH]
  - final = lhsT=outT slice, rhs=Wo  (+ bias, DVE)    out  [CH, 1024] -> DRAM bf16
"""

import os
import sys

for _p in ("/opt/pypackages", "/opt/trn_rl_repo", "/root/.axon_site/_ro/trn_rl_repo",
           "/root/.axon_site/_ro/pypackages"):
    if os.path.isdir(_p) and _p not in sys.path:
        sys.path.append(_p)

import numpy as np

B = 8
NQ = 4096
NK = 77
QD = 1024   # query feature dim
CD = 768    # context feature dim
ID = 1024   # inner dim (= H * D)
H = 16
D = 64
SCALE = D ** -0.5
CH = 512    # seq chunk per pipeline iteration
NCHUNK = NQ // CH
P = 128
NK2 = 78  # NK padded even for fp32r moving/dst

_STATE: dict = {}


def _build():
    import concourse.bass as bass
    import concourse.tile as tile
    from concourse import bacc, mybir
    from concourse.masks import make_identity

    F32 = mybir.dt.float32
    F32R = mybir.dt.float32r
    BF16 = mybir.dt.bfloat16
    AF = mybir.ActivationFunctionType
    ALU = mybir.AluOpType

    nc = bacc.Bacc("TRN2", target_bir_lowering=False, debug=False)

    x_d = nc.dram_tensor("x", [NQ, QD], BF16, kind="ExternalInput").ap()
    ctx_d = nc.dram_tensor("context", [NK, CD], BF16, kind="ExternalInput").ap()
    wq_d = nc.dram_tensor("Wq", [QD, ID], BF16, kind="ExternalInput").ap()
    wk_d = nc.dram_tensor("Wk", [CD, ID], BF16, kind="ExternalInput").ap()
    wv_d = nc.dram_tensor("Wv", [CD, ID], BF16, kind="ExternalInput").ap()
    wo_d = nc.dram_tensor("Wo", [ID, QD], BF16, kind="ExternalInput").ap()
    bo_d = nc.dram_tensor("bo", [QD], F32, kind="ExternalInput").ap()
    out_d = nc.dram_tensor("out", [NQ, QD], BF16, kind="ExternalOutput").ap()

    KQ = QD // P   # 8 k-tiles for x/Wq
    KC = CD // P   # 6 k-tiles for context/Wk/Wv
    KO = ID // P   # 8 k-tiles for Wo

    with tile.TileContext(nc) as tc:
        with (
            tc.tile_pool(name="singles", bufs=1) as singles,
            tc.tile_pool(name="xn_pool", bufs=CH // P + 1) as xn_pool,
            tc.tile_pool(name="wstage", bufs=2) as wstage_pool,
            tc.tile_pool(name="xt_pool", bufs=KQ) as xt_pool,
            tc.tile_pool(name="qt_pool", bufs=KQ) as qt_pool,
            tc.tile_pool(name="ot_pool", bufs=KO) as ot_pool,
            tc.tile_pool(name="expt_pool", bufs=3) as expt_pool,
            tc.tile_pool(name="recip_pool", bufs=2) as recip_pool,
            tc.tile_pool(name="fin_pool", bufs=2) as fin_pool,
            tc.tile_pool(name="ps_small", bufs=4, space="PSUM") as ps_small,
            tc.tile_pool(name="ps_q", bufs=2, space="PSUM") as ps_q,
            tc.tile_pool(name="ps_wo", bufs=2, space="PSUM") as ps_wo,
        ):
            # ---------------- one-time setup ----------------
            ident = singles.tile([P, P], F32, tag="ident")
            make_identity(nc, ident)
            identb = singles.tile([P, P], BF16, tag="identb")
            nc.vector.tensor_copy(identb[:, :], ident[:, :])

            # ones row for broadcasting per-head 1/denom across 64 partitions
            ones_f32 = singles.tile([NK, D], F32, tag="ones_f32")
            nc.gpsimd.memset(ones_f32[:, :], 1.0)
            ones_col = singles.tile([1, D], F32R, tag="ones_col")
            nc.vector.tensor_copy(ones_col[:, :], ones_f32[0:1, :])

            # bias broadcast to all 128 partitions via partition-step-0 DMA
            bias_sb = singles.tile([P, QD], F32, tag="bias")
            bo_bcast = bass.AP(
                tensor=bo_d.tensor, offset=bo_d.offset,
                ap=[[0, P], list(bo_d.ap[0])],
            )
            nc.gpsimd.dma_start(out=bias_sb[:, :], in_=bo_bcast)

            # weights: DMA bf16 staging, then widening copy into fp32r tiles
            wq_sb = [singles.tile([P, ID], F32R, tag=f"wq{k}", name=f"wq{k}") for k in range(KQ)]
            for k in range(KQ):
                stg = wstage_pool.tile([P, ID], BF16, tag="wstage", name="wstage")
                nc.sync.dma_start(out=stg[:, :], in_=wq_d[k * P:(k + 1) * P, :])
                nc.vector.tensor_copy(wq_sb[k][:, :], stg[:, :])
            wk_sb = [singles.tile([P, ID], F32R, tag=f"wk{k}", name=f"wk{k}") for k in range(KC)]
            for k in range(KC):
                stg = wstage_pool.tile([P, ID], BF16, tag="wstage", name="wstage")
                nc.sync.dma_start(out=stg[:, :], in_=wk_d[k * P:(k + 1) * P, :])
                nc.vector.tensor_copy(wk_sb[k][:, :], stg[:, :])
            wv_sb = [singles.tile([P, ID], F32R, tag=f"wv{k}", name=f"wv{k}") for k in range(KC)]
            for k in range(KC):
                stg = wstage_pool.tile([P, ID], BF16, tag="wstage", name="wstage")
                nc.sync.dma_start(out=stg[:, :], in_=wv_d[k * P:(k + 1) * P, :])
                nc.vector.tensor_copy(wv_sb[k][:, :], stg[:, :])
            wo_sb = [singles.tile([P, QD], F32R, tag=f"wo{k}", name=f"wo{k}") for k in range(KO)]
            for k in range(KO):
                stg = wstage_pool.tile([P, QD], BF16, tag="wstage", name="wstage")
                nc.sync.dma_start(out=stg[:, :], in_=wo_d[k * P:(k + 1) * P, :])
                nc.vector.tensor_copy(wo_sb[k][:, :], stg[:, :])

            # context: load natural, transpose to cT tiles [128, 77] x 6
            ctx_sb = singles.tile([NK, CD], BF16, tag="ctx")
            nc.sync.dma_start(out=ctx_sb[:, :], in_=ctx_d[:, :])
            zeros_f32 = singles.tile([P, 1], F32, tag="zeros_f32")
            nc.gpsimd.memset(zeros_f32[:, :], 0.0)
            ct_sb = [singles.tile([P, NK2], F32R, tag=f"ct{k}", name=f"ct{k}") for k in range(KC)]
            for k in range(KC):
                pt = ps_small.tile([P, NK], BF16, tag="ps_attn")
                nc.tensor.transpose(pt[:, :], ctx_sb[:, k * P:(k + 1) * P],
                                    identb[0:NK, 0:NK])
                nc.vector.tensor_copy(ct_sb[k][:, 0:NK], pt[:, :])
                nc.vector.tensor_copy(ct_sb[k][:, NK:NK2], zeros_f32[:, :])

            # kT tiles [128, 77] x 8 (inner dim on partitions)
            kt_sb = [singles.tile([P, NK2], F32R, tag=f"kt{m}", name=f"kt{m}") for m in range(KQ)]
            for m in range(KQ):
                pk = ps_small.tile([P, NK2], F32, tag="ps_attn")
                for k in range(KC):
                    nc.tensor.matmul(
                        pk[:, :], wk_sb[k][:, m * P:(m + 1) * P], ct_sb[k][:, :],
                        start=(k == 0), stop=(k == KC - 1))
                nc.vector.tensor_copy(kt_sb[m][:, :], pk[:, :])

            # v natural [77, 1024] into v_aug [77, 16*65] with ones col per head
            v_aug = singles.tile([NK, H * (D + 1)], F32R, tag="vaug")
            for h in range(H):
                nc.vector.tensor_copy(
                    v_aug[:, h * (D + 1) + D: (h + 1) * (D + 1)], ones_f32[:, 0:1])
            for n in range(2):
                pv = ps_wo.tile([NK, 512], F32, tag="ps_wo")
                for k in range(KC):
                    nc.tensor.matmul(
                        pv[:, :], ct_sb[k][:, 0:NK], wv_sb[k][:, n * 512:(n + 1) * 512],
                        start=(k == 0), stop=(k == KC - 1))
                for hh in range(8):
                    h = n * 8 + hh
                    nc.vector.tensor_copy(
                        v_aug[:, h * (D + 1): h * (D + 1) + D],
                        pv[:, hh * D:(hh + 1) * D])

            # ---------------- main loop over seq chunks ----------------
            for c in range(NCHUNK):
                # load x natural: CH rows of x -> CH//P tiles [128, QD]
                xn = []
                for s in range(CH // P):
                    t = xn_pool.tile([P, QD], BF16, tag="xn", name="xn")
                    nc.sync.dma_start(
                        out=t[:, :],
                        in_=x_d[c * CH + s * P: c * CH + (s + 1) * P, :])
                    xn.append(t)

                # transpose to xT tiles [128, CH] x 8; one wide PSUM evict per tile
                xt = []
                for k in range(KQ):
                    t = xt_pool.tile([P, CH], F32R, tag="xt", name="xt")
                    pt = ps_small.tile([P, CH], BF16, tag="ps_attn")
                    for s in range(CH // P):
                        nc.tensor.transpose(
                            pt[:, s * P:(s + 1) * P], xn[s][:, k * P:(k + 1) * P],
                            identb[:, :])
                    nc.vector.tensor_copy(t[:, :], pt[:, :])
                    xt.append(t)

                # qT tiles [128, CH] x 8
                qt = []
                for m in range(KQ):
                    pq = ps_q.tile([P, CH], F32, tag="ps_q")
                    for k in range(KQ):
                        nc.tensor.matmul(
                            pq[:, :], wq_sb[k][:, m * P:(m + 1) * P], xt[k][:, :],
                            start=(k == 0), stop=(k == KQ - 1))
                    t = qt_pool.tile([P, CH], F32R, tag="qt")
                    nc.vector.tensor_copy(t[:, :], pq[:, :])
                    qt.append(t)

                # attention per head-pair
                ot = [ot_pool.tile([P, CH], F32R, tag="ot", name="ot") for _ in range(KO)]
                for h in range(H):
                    mt = h // 2   # which kT/qT tile
                    lo = (h % 2) * D
                    psim = ps_small.tile([NK, CH], F32, tag="ps_attn")
                    nc.tensor.matmul(
                        psim[:, :],
                        kt_sb[mt][lo:lo + D, 0:NK], qt[mt][lo:lo + D, :],
                        start=True, stop=True)
                    et = expt_pool.tile([NK, CH], F32R, tag="expt")
                    nc.scalar.activation(et[:, :], psim[:, :], AF.Exp,
                                         scale=float(SCALE))
                    pav = ps_small.tile([D + 1, CH], F32, tag="ps_attn")
                    nc.tensor.matmul(
                        pav[:, :],
                        v_aug[:, h * (D + 1): (h + 1) * (D + 1)], et[:, :],
                        start=True, stop=True)
                    rc = recip_pool.tile([1, CH], F32R, tag="recip")
                    with nc.allow_low_precision(reason="fp32r rounding of 1/denom"):
                        nc.vector.reciprocal(rc[:, :], pav[D:D + 1, :])
                    # broadcast 1/denom across 64 partitions via K=1 matmul
                    pb = ps_small.tile([D, CH], F32, tag="ps_attn")
                    nc.tensor.matmul(pb[:, :], ones_col[:, :], rc[:, :],
                                     start=True, stop=True)
                    pb_sb = recip_pool.tile([D, CH], F32, tag="pb_sb", name="pb_sb")
                    nc.vector.tensor_copy(pb_sb[:, :], pb[:, :])
                    nc.vector.tensor_tensor(
                        ot[mt][lo:lo + D, :],
                        pav[0:D, :], pb_sb[:, :], op=ALU.mult)

                # output projection + bias
                for s in range(CH // P):
                    for n in range(QD // 512):
                        po = ps_wo.tile([P, 512], F32, tag="ps_wo")
                        for k in range(KO):
                            nc.tensor.matmul(
                                po[:, :],
                                ot[k][:, s * P:(s + 1) * P],
                                wo_sb[k][:, n * 512:(n + 1) * 512],
                                start=(k == 0), stop=(k == KO - 1))
                        ft = fin_pool.tile([P, 512], BF16, tag="fin")
                        nc.vector.tensor_tensor(
                            ft[:, :], po[:, :], bias_sb[:, n * 512:(n + 1) * 512],
                            op=ALU.add)
                        nc.sync.dma_start(
                            out=out_d[c * CH + s * P: c * CH + (s + 1) * P,
                                      n * 512:(n + 1) * 512],
                            in_=ft[:, :])

    nc.compile()
    return nc


# DRAM input order must match _build's dram_tensor creation order.
_IN_NAMES = ("x", "context", "Wq", "Wk", "Wv", "Wo", "bo")
# Which inputs are sharded over cores (axis 0) vs replicated to all cores.
_SHARDED = {"x", "context"}


def _ensure_compiled():
    """Build the Bass module and AOT-compile the sharded executable once."""
    if "compiled" in _STATE:
        return _STATE

    import jax
    import ml_dtypes
    from jax.sharding import Mesh, PartitionSpec, NamedSharding

    try:
        from jax import shard_map
    except ImportError:
        from jax.experimental.shard_map import shard_map

    from concourse import bass2jax, mybir
    from concourse.bass2jax import _bass_exec_p, install_neuronx_cc_hook

    install_neuronx_cc_hook()

    nc = _build()

    devices = jax.devices()[:B]
    mesh = Mesh(np.asarray(devices), ("core",))
    core = NamedSharding(mesh, PartitionSpec("core"))
    rep = NamedSharding(mesh, PartitionSpec())

    # Bacc implicitly declares a partition_id ExternalInput; it must be the
    # LAST bass_exec operand (generated on-device via PartitionIdOp), and its
    # name must ride last in in_names — exactly what run_bass_via_pjrt does.
    partition_name = nc.partition_id_tensor.name if nc.partition_id_tensor else None
    out_avals = []
    out_names = []
    for alloc in nc.m.functions[0].allocations:
        if not isinstance(alloc, mybir.MemoryLocationSet):
            continue
        if alloc.kind == "ExternalOutput":
            out_names.append(alloc.memorylocations[0].name)
            out_avals.append(jax.core.ShapedArray(
                tuple(alloc.tensor_shape), mybir.dt.np(alloc.dtype)))
    all_in = list(_IN_NAMES) + list(out_names)
    if partition_name is not None:
        all_in.append(partition_name)

    def _body(*args):
        operands = list(args)
        if partition_name is not None:
            operands.append(bass2jax.partition_id_tensor())
        return tuple(_bass_exec_p.bind(
            *operands,
            out_avals=tuple(out_avals),
            in_names=tuple(all_in),
            out_names=tuple(out_names),
            lowering_input_output_aliases=(),
            sim_require_finite=True,
            sim_require_nnan=True,
            nc=nc,
        ))

    n_in = len(_IN_NAMES)
    in_specs = tuple(
        PartitionSpec("core") if nm in _SHARDED else PartitionSpec()
        for nm in _IN_NAMES
    ) + (PartitionSpec("core"),)
    try:
        smapped = shard_map(_body, mesh=mesh, in_specs=in_specs,
                            out_specs=(PartitionSpec("core"),), check_vma=False)
    except TypeError:
        smapped = shard_map(_body, mesh=mesh, in_specs=in_specs,
                            out_specs=(PartitionSpec("core"),), check_rep=False)
    sharded = jax.jit(
        smapped,
        donate_argnums=(n_in,),
        keep_unused=True,
    )

    bf16 = ml_dtypes.bfloat16
    arg_structs = []
    for nm in _IN_NAMES:
        if nm == "x":
            arg_structs.append(jax.ShapeDtypeStruct((B * NQ, QD), bf16, sharding=core))
        elif nm == "context":
            arg_structs.append(jax.ShapeDtypeStruct((B * NK, CD), bf16, sharding=core))
        elif nm == "Wq":
            arg_structs.append(jax.ShapeDtypeStruct((QD, ID), bf16, sharding=rep))
        elif nm in ("Wk", "Wv"):
            arg_structs.append(jax.ShapeDtypeStruct((CD, ID), bf16, sharding=rep))
        elif nm == "Wo":
            arg_structs.append(jax.ShapeDtypeStruct((ID, QD), bf16, sharding=rep))
        elif nm == "bo":
            arg_structs.append(jax.ShapeDtypeStruct((QD,), np.float32, sharding=rep))
    arg_structs.append(jax.ShapeDtypeStruct((B * NQ, QD), bf16, sharding=core))

    compiled = sharded.lower(*arg_structs).compile()

    _STATE.update(dict(compiled=compiled, mesh=mesh, core=core, rep=rep))
    return _STATE


def _run(inputs):
    """Enqueue async uploads, compile while they stream, execute, fetch."""
    import time
    from concurrent.futures import ThreadPoolExecutor

    import jax
    import ml_dtypes
    from jax.sharding import Mesh, PartitionSpec, NamedSharding

    bf16 = ml_dtypes.bfloat16
    devices = jax.devices()[:B]
    assert len(devices) == B

    mesh = Mesh(np.asarray(devices), ("core",))
    core = NamedSharding(mesh, PartitionSpec("core"))
    rep = NamedSharding(mesh, PartitionSpec())

    dbg = bool(os.environ.get("BASSK_DEBUG"))
    tlog = []

    def _t(label, t0):
        if dbg:
            tlog.append(f"{label}: {time.time() - t0:.2f}s")

    # device_put is async — enqueue everything on the main thread BEFORE
    # compiling (a first-time upload racing the compile RPCs hits a 50s+
    # pathological path in the tunnel); transfers then stream while the
    # main thread builds + compiles.
    t0 = time.time()
    puts: dict = {}
    x = np.asarray(inputs["x"]).astype(bf16).reshape(B * NQ, QD)
    puts["x"] = jax.device_put(x, core)
    ctx = np.asarray(inputs["context"]).astype(bf16).reshape(B * NK, CD)
    puts["context"] = jax.device_put(ctx, core)
    for nm in ("Wq", "Wk", "Wv", "Wo"):
        puts[nm] = jax.device_put(
            np.ascontiguousarray(np.asarray(inputs[nm]).astype(bf16)), rep)
    puts["bo"] = jax.device_put(
        np.ascontiguousarray(np.asarray(inputs["bo"], dtype=np.float32)), rep)
    puts["out0"] = jax.device_put(np.zeros((B * NQ, QD), bf16), core)
    _t("enqueue_puts", t0)

    t0 = time.time()
    st = _ensure_compiled()
    _t("build_compile", t0)
    t0 = time.time()
    jax.block_until_ready(list(puts.values()))
    _t("upload_drain", t0)

    args = [puts[nm] for nm in _IN_NAMES] + [puts["out0"]]
    t0 = time.time()
    (out,) = st["compiled"](*args)
    jax.block_until_ready(out)
    exec_s = time.time() - t0
    _t("exec", t0)

    t0 = time.time()
    res = np.empty((B * NQ, QD), np.float32)

    def _fetch(s):
        res[s.index] = np.asarray(s.data, dtype=np.float32)

    shards = list(out.addressable_shards)
    with ThreadPoolExecutor(max_workers=8) as ex:
        list(ex.map(_fetch, shards))
    _t("fetch", t0)
    if dbg:
        print("[kernel timing] " + "  ".join(tlog), flush=True)
    return res.reshape(B, NQ, QD), exec_s


def run(inputs, trace=False):
    """Returns (output, device dispatch+exec seconds)."""
    out, exec_s = _run(inputs)
    return out, exec_s


def kernel(**inputs) -> np.ndarray:
    out, _ = _run(inputs)
    return out


# revision 13
# speedup vs baseline: 16.4888x; 16.4888x over previous
"""Trainium2 Bass kernel for CrossAttention (B=8, Nq=4096, Nk=77, H=16, D=64).

Sharding: data-parallel over batch — one batch element per NeuronCore (8 cores).

End-to-end latency strategy (the dominant cost is host<->device transfer over
the PJRT tunnel plus compile, not kernel exec):
  - All big DRAM I/O is bf16: x upload 64 MB, out download 64 MB.
  - Weights are shipped ONCE (replicated PartitionSpec()) instead of 8 copies.
  - Uploads stream on a worker thread while the main thread builds the Bass
    module and jit-compiles the sharded executable.
  - Output is fetched per-shard in parallel threads, then upcast to f32.

Per-core dataflow (PE matmuls in f32r; bf16 only at the DMA boundary):
  - transpose x chunk on PE (identity matmul)         xT   [1024, CH]
  - qT = Wq^T-free matmul: lhsT=Wq[k,m], rhs=xT[k]    qT   [1024, CH]
  - kT = lhsT=Wk slice, rhs=cT (context transposed)   kT   [1024, 77]
  - v  = lhsT=cT, rhs=Wv (natural layout)             v    [77, 1024] (+ ones col per head)
  - simT_h = lhsT=kT_h [64,77], rhs=qT_h [64,CH]      simT [77, CH]
  - expT_h = exp(scale*simT) on ACT                   expT [77, CH]
  - avT_h  = lhsT=v_aug_h [77,65], rhs=expT           avT  [65, CH] (row 64 = softmax denom)
  - recip + broadcast via tiny matmul, DVE multiply   outT [1024, CH]
  - final = lhsT=outT slice, rhs=Wo  (+ bias, DVE)    out  [CH, 1024] -> DRAM bf16
"""

import os
import sys

for _p in ("/opt/pypackages", "/opt/trn_rl_repo", "/root/.axon_site/_ro/trn_rl_repo",
           "/root/.axon_site/_ro/pypackages"):
    if os.path.isdir(_p) and _p not in sys.path:
        sys.path.append(_p)

import numpy as np

B = 8
NQ = 4096
NK = 77
QD = 1024   # query feature dim
CD = 768    # context feature dim
ID = 1024   # inner dim (= H * D)
H = 16
D = 64
SCALE = D ** -0.5
CH = 512    # seq chunk per pipeline iteration
NCHUNK = NQ // CH
P = 128
NK2 = 78  # NK padded even for fp32r moving/dst

_STATE: dict = {}


def _build():
    import concourse.bass as bass
    import concourse.tile as tile
    from concourse import bacc, mybir
    from concourse.masks import make_identity

    F32 = mybir.dt.float32
    F32R = mybir.dt.float32r
    BF16 = mybir.dt.bfloat16
    AF = mybir.ActivationFunctionType
    ALU = mybir.AluOpType

    nc = bacc.Bacc("TRN2", target_bir_lowering=False, debug=False)

    x_d = nc.dram_tensor("x", [NQ, QD], BF16, kind="ExternalInput").ap()
    ctx_d = nc.dram_tensor("context", [NK, CD], BF16, kind="ExternalInput").ap()
    wq_d = nc.dram_tensor("Wq", [QD, ID], BF16, kind="ExternalInput").ap()
    wk_d = nc.dram_tensor("Wk", [CD, ID], BF16, kind="ExternalInput").ap()
    wv_d = nc.dram_tensor("Wv", [CD, ID], BF16, kind="ExternalInput").ap()
    wo_d = nc.dram_tensor("Wo", [ID, QD], BF16, kind="ExternalInput").ap()
    bo_d = nc.dram_tensor("bo", [QD], F32, kind="ExternalInput").ap()
    out_d = nc.dram_tensor("out", [NQ, QD], BF16, kind="ExternalOutput").ap()

    KQ = QD // P   # 8 k-tiles for x/Wq
    KC = CD // P   # 6 k-tiles for context/Wk/Wv
    KO = ID // P   # 8 k-tiles for Wo

    with tile.TileContext(nc) as tc:
        with (
            tc.tile_pool(name="singles", bufs=1) as singles,
            tc.tile_pool(name="xn_pool", bufs=CH // P + 1) as xn_pool,
            tc.tile_pool(name="wstage", bufs=2) as wstage_pool,
            tc.tile_pool(name="xt_pool", bufs=KQ) as xt_pool,
            tc.tile_pool(name="qt_pool", bufs=KQ) as qt_pool,
            tc.tile_pool(name="ot_pool", bufs=KO) as ot_pool,
            tc.tile_pool(name="expt_pool", bufs=3) as expt_pool,
            tc.tile_pool(name="recip_pool", bufs=2) as recip_pool,
            tc.tile_pool(name="fin_pool", bufs=2) as fin_pool,
            tc.tile_pool(name="ps_small", bufs=4, space="PSUM") as ps_small,
            tc.tile_pool(name="ps_q", bufs=2, space="PSUM") as ps_q,
            tc.tile_pool(name="ps_wo", bufs=2, space="PSUM") as ps_wo,
        ):
            # ---------------- one-time setup ----------------
            ident = singles.tile([P, P], F32, tag="ident")
            make_identity(nc, ident)
            identb = singles.tile([P, P], BF16, tag="identb")
            nc.vector.tensor_copy(identb[:, :], ident[:, :])

            # ones row for broadcasting per-head 1/denom across 64 partitions
            ones_f32 = singles.tile([NK, D], F32, tag="ones_f32")
            nc.gpsimd.memset(ones_f32[:, :], 1.0)
            ones_col = singles.tile([1, D], F32R, tag="ones_col")
            nc.vector.tensor_copy(ones_col[:, :], ones_f32[0:1, :])

            # bias broadcast to all 128 partitions via partition-step-0 DMA
            bias_sb = singles.tile([P, QD], F32, tag="bias")
            bo_bcast = bass.AP(
                tensor=bo_d.tensor, offset=bo_d.offset,
                ap=[[0, P], list(bo_d.ap[0])],
            )
            nc.gpsimd.dma_start(out=bias_sb[:, :], in_=bo_bcast)

            # weights: DMA bf16 staging, then widening copy into fp32r tiles
            wq_sb = [singles.tile([P, ID], F32R, tag=f"wq{k}", name=f"wq{k}") for k in range(KQ)]
            for k in range(KQ):
                stg = wstage_pool.tile([P, ID], BF16, tag="wstage", name="wstage")
                nc.sync.dma_start(out=stg[:, :], in_=wq_d[k * P:(k + 1) * P, :])
                nc.vector.tensor_copy(wq_sb[k][:, :], stg[:, :])
            wk_sb = [singles.tile([P, ID], F32R, tag=f"wk{k}", name=f"wk{k}") for k in range(KC)]
            for k in range(KC):
                stg = wstage_pool.tile([P, ID], BF16, tag="wstage", name="wstage")
                nc.sync.dma_start(out=stg[:, :], in_=wk_d[k * P:(k + 1) * P, :])
                nc.vector.tensor_copy(wk_sb[k][:, :], stg[:, :])
            wv_sb = [singles.tile([P, ID], F32R, tag=f"wv{k}", name=f"wv{k}") for k in range(KC)]
            for k in range(KC):
                stg = wstage_pool.tile([P, ID], BF16, tag="wstage", name="wstage")
                nc.sync.dma_start(out=stg[:, :], in_=wv_d[k * P:(k + 1) * P, :])
                nc.vector.tensor_copy(wv_sb[k][:, :], stg[:, :])
            wo_sb = [singles.tile([P, QD], F32R, tag=f"wo{k}", name=f"wo{k}") for k in range(KO)]
            for k in range(KO):
                stg = wstage_pool.tile([P, QD], BF16, tag="wstage", name="wstage")
                nc.sync.dma_start(out=stg[:, :], in_=wo_d[k * P:(k + 1) * P, :])
                nc.vector.tensor_copy(wo_sb[k][:, :], stg[:, :])

            # context: load natural, transpose to cT tiles [128, 77] x 6
            ctx_sb = singles.tile([NK, CD], BF16, tag="ctx")
            nc.sync.dma_start(out=ctx_sb[:, :], in_=ctx_d[:, :])
            zeros_f32 = singles.tile([P, 1], F32, tag="zeros_f32")
            nc.gpsimd.memset(zeros_f32[:, :], 0.0)
            ct_sb = [singles.tile([P, NK2], F32R, tag=f"ct{k}", name=f"ct{k}") for k in range(KC)]
            for k in range(KC):
                pt = ps_small.tile([P, NK], BF16, tag="ps_attn")
                nc.tensor.transpose(pt[:, :], ctx_sb[:, k * P:(k + 1) * P],
                                    identb[0:NK, 0:NK])
                nc.vector.tensor_copy(ct_sb[k][:, 0:NK], pt[:, :])
                nc.vector.tensor_copy(ct_sb[k][:, NK:NK2], zeros_f32[:, :])

            # kT tiles [128, 77] x 8 (inner dim on partitions)
            kt_sb = [singles.tile([P, NK2], F32R, tag=f"kt{m}", name=f"kt{m}") for m in range(KQ)]
            for m in range(KQ):
                pk = ps_small.tile([P, NK2], F32, tag="ps_attn")
                for k in range(KC):
                    nc.tensor.matmul(
                        pk[:, :], wk_sb[k][:, m * P:(m + 1) * P], ct_sb[k][:, :],
                        start=(k == 0), stop=(k == KC - 1))
                nc.vector.tensor_copy(kt_sb[m][:, :], pk[:, :])

            # v natural [77, 1024] into v_aug [77, 16*65] with ones col per head
            v_aug = singles.tile([NK, H * (D + 1)], F32R, tag="vaug")
            for h in range(H):
                nc.vector.tensor_copy(
                    v_aug[:, h * (D + 1) + D: (h + 1) * (D + 1)], ones_f32[:, 0:1])
            for n in range(2):
                pv = ps_wo.tile([NK, 512], F32, tag="ps_wo")
                for k in range(KC):
                    nc.tensor.matmul(
                        pv[:, :], ct_sb[k][:, 0:NK], wv_sb[k][:, n * 512:(n + 1) * 512],
                        start=(k == 0), stop=(k == KC - 1))
                for hh in range(8):
                    h = n * 8 + hh
                    nc.vector.tensor_copy(
                        v_aug[:, h * (D + 1): h * (D + 1) + D],
                        pv[:, hh * D:(hh + 1) * D])

            # ---------------- main loop over seq chunks ----------------
            for c in range(NCHUNK):
                # load x natural: CH rows of x -> CH//P tiles [128, QD]
                xn = []
                for s in range(CH // P):
                    t = xn_pool.tile([P, QD], BF16, tag="xn", name="xn")
                    nc.sync.dma_start(
                        out=t[:, :],
                        in_=x_d[c * CH + s * P: c * CH + (s + 1) * P, :])
                    xn.append(t)

                # transpose to xT tiles [128, CH] x 8; one wide PSUM evict per tile
                xt = []
                for k in range(KQ):
                    t = xt_pool.tile([P, CH], F32R, tag="xt", name="xt")
                    pt = ps_small.tile([P, CH], BF16, tag="ps_attn")
                    for s in range(CH // P):
                        nc.tensor.transpose(
                            pt[:, s * P:(s + 1) * P], xn[s][:, k * P:(k + 1) * P],
                            identb[:, :])
                    nc.vector.tensor_copy(t[:, :], pt[:, :])
                    xt.append(t)

                # qT tiles [128, CH] x 8
                qt = []
                for m in range(KQ):
                    pq = ps_q.tile([P, CH], F32, tag="ps_q")
                    for k in range(KQ):
                        nc.tensor.matmul(
                            pq[:, :], wq_sb[k][:, m * P:(m + 1) * P], xt[k][:, :],
                            start=(k == 0), stop=(k == KQ - 1))
                    t = qt_pool.tile([P, CH], F32R, tag="qt")
                    nc.vector.tensor_copy(t[:, :], pq[:, :])
                    qt.append(t)

                # attention per head-pair
                ot = [ot_pool.tile([P, CH], F32R, tag="ot", name="ot") for _ in range(KO)]
                for h in range(H):
                    mt = h // 2   # which kT/qT tile
                    lo = (h % 2) * D
                    psim = ps_small.tile([NK, CH], F32, tag="ps_attn")
                    nc.tensor.matmul(
                        psim[:, :],
                        kt_sb[mt][lo:lo + D, 0:NK], qt[mt][lo:lo + D, :],
                        start=True, stop=True)
                    et = expt_pool.tile([NK, CH], F32R, tag="expt")
                    nc.scalar.activation(et[:, :], psim[:, :], AF.Exp,
                                         scale=float(SCALE))
                    pav = ps_small.tile([D + 1, CH], F32, tag="ps_attn")
                    nc.tensor.matmul(
                        pav[:, :],
                        v_aug[:, h * (D + 1): (h + 1) * (D + 1)], et[:, :],
                        start=True, stop=True)
                    rc = recip_pool.tile([1, CH], F32R, tag="recip")
                    with nc.allow_low_precision(reason="fp32r rounding of 1/denom"):
                        nc.vector.reciprocal(rc[:, :], pav[D:D + 1, :])
                    # broadcast 1/denom across 64 partitions via K=1 matmul
                    pb = ps_small.tile([D, CH], F32, tag="ps_attn")
                    nc.tensor.matmul(pb[:, :], ones_col[:, :], rc[:, :],
                                     start=True, stop=True)
                    pb_sb = recip_pool.tile([D, CH], F32, tag="pb_sb", name="pb_sb")
                    nc.vector.tensor_copy(pb_sb[:, :], pb[:, :])
                    nc.vector.tensor_tensor(
                        ot[mt][lo:lo + D, :],
                        pav[0:D, :], pb_sb[:, :], op=ALU.mult)

                # output projection + bias
                for s in range(CH // P):
                    for n in range(QD // 512):
                        po = ps_wo.tile([P, 512], F32, tag="ps_wo")
                        for k in range(KO):
                            nc.tensor.matmul(
                                po[:, :],
                                ot[k][:, s * P:(s + 1) * P],
                                wo_sb[k][:, n * 512:(n + 1) * 512],
                                start=(k == 0), stop=(k == KO - 1))
                        ft = fin_pool.tile([P, 512], BF16, tag="fin")
                        nc.vector.tensor_tensor(
                            ft[:, :], po[:, :], bias_sb[:, n * 512:(n + 1) * 512],
                            op=ALU.add)
                        nc.sync.dma_start(
                            out=out_d[c * CH + s * P: c * CH + (s + 1) * P,
                                      n * 512:(n + 1) * 512],
                            in_=ft[:, :])

    nc.compile()
    return nc


# DRAM input order must match _build's dram_tensor creation order.
_IN_NAMES = ("x", "context", "Wq", "Wk", "Wv", "Wo", "bo")
# Which inputs are sharded over cores (axis 0) vs replicated to all cores.
_SHARDED = {"x", "context"}


def _ensure_compiled():
    """Build the Bass module and AOT-compile the sharded executable once."""
    if "compiled" in _STATE:
        return _STATE

    import jax
    import ml_dtypes
    from jax.sharding import Mesh, PartitionSpec, NamedSharding

    try:
        from jax import shard_map
    except ImportError:
        from jax.experimental.shard_map import shard_map

    from concourse import bass2jax, mybir
    from concourse.bass2jax import _bass_exec_p, install_neuronx_cc_hook

    install_neuronx_cc_hook()

    nc = _build()

    devices = jax.devices()[:B]
    mesh = Mesh(np.asarray(devices), ("core",))
    core = NamedSharding(mesh, PartitionSpec("core"))
    rep = NamedSharding(mesh, PartitionSpec())

    # Bacc implicitly declares a partition_id ExternalInput; it must be the
    # LAST bass_exec operand (generated on-device via PartitionIdOp), and its
    # name must ride last in in_names — exactly what run_bass_via_pjrt does.
    partition_name = nc.partition_id_tensor.name if nc.partition_id_tensor else None
    out_avals = []
    out_names = []
    for alloc in nc.m.functions[0].allocations:
        if not isinstance(alloc, mybir.MemoryLocationSet):
            continue
        if alloc.kind == "ExternalOutput":
            out_names.append(alloc.memorylocations[0].name)
            out_avals.append(jax.core.ShapedArray(
                tuple(alloc.tensor_shape), mybir.dt.np(alloc.dtype)))
    all_in = list(_IN_NAMES) + list(out_names)
    if partition_name is not None:
        all_in.append(partition_name)

    def _body(*args):
        operands = list(args)
        if partition_name is not None:
            operands.append(bass2jax.partition_id_tensor())
        return tuple(_bass_exec_p.bind(
            *operands,
            out_avals=tuple(out_avals),
            in_names=tuple(all_in),
            out_names=tuple(out_names),
            lowering_input_output_aliases=(),
            sim_require_finite=True,
            sim_require_nnan=True,
            nc=nc,
        ))

    n_in = len(_IN_NAMES)
    in_specs = tuple(
        PartitionSpec("core") if nm in _SHARDED else PartitionSpec()
        for nm in _IN_NAMES
    ) + (PartitionSpec("core"),)
    try:
        smapped = shard_map(_body, mesh=mesh, in_specs=in_specs,
                            out_specs=(PartitionSpec("core"),), check_vma=False)
    except TypeError:
        smapped = shard_map(_body, mesh=mesh, in_specs=in_specs,
                            out_specs=(PartitionSpec("core"),), check_rep=False)
    sharded = jax.jit(
        smapped,
        donate_argnums=(n_in,),
        keep_unused=True,
    )

    bf16 = ml_dtypes.bfloat16
    arg_structs = []
    for nm in _IN_NAMES:
        if nm == "x":
            arg_structs.append(jax.ShapeDtypeStruct((B * NQ, QD), bf16, sharding=core))
        elif nm == "context":
            arg_structs.append(jax.ShapeDtypeStruct((B * NK, CD), bf16, sharding=core))
        elif nm == "Wq":
            arg_structs.append(jax.ShapeDtypeStruct((QD, ID), bf16, sharding=rep))
        elif nm in ("Wk", "Wv"):
            arg_structs.append(jax.ShapeDtypeStruct((CD, ID), bf16, sharding=rep))
        elif nm == "Wo":
            arg_structs.append(jax.ShapeDtypeStruct((ID, QD), bf16, sharding=rep))
        elif nm == "bo":
            arg_structs.append(jax.ShapeDtypeStruct((QD,), np.float32, sharding=rep))
    arg_structs.append(jax.ShapeDtypeStruct((B * NQ, QD), bf16, sharding=core))

    compiled = sharded.lower(*arg_structs).compile()

    _STATE.update(dict(compiled=compiled, mesh=mesh, core=core, rep=rep))
    return _STATE


def _run(inputs):
    """Enqueue async uploads, compile while they stream, execute, fetch."""
    import time
    from concurrent.futures import ThreadPoolExecutor

    import jax
    import ml_dtypes
    from jax.sharding import Mesh, PartitionSpec, NamedSharding

    bf16 = ml_dtypes.bfloat16
    devices = jax.devices()[:B]
    assert len(devices) == B

    mesh = Mesh(np.asarray(devices), ("core",))
    core = NamedSharding(mesh, PartitionSpec("core"))
    rep = NamedSharding(mesh, PartitionSpec())

    dbg = bool(os.environ.get("BASSK_DEBUG"))
    tlog = []

    def _t(label, t0):
        if dbg:
            tlog.append(f"{label}: {time.time() - t0:.2f}s")

    # Upload via raw per-device puts + make_array_from_single_device_arrays.
    # NamedSharding device_puts route through a resharding program that
    # intermittently hits a 1-2 min pathological path on cold tunnels; plain
    # single-device transfers have never shown it. Enqueue before compiling.
    t0 = time.time()
    puts: dict = {}

    def _put_sharded(arr2d):
        n = arr2d.shape[0] // B
        shards = [jax.device_put(arr2d[i * n:(i + 1) * n], d)
                  for i, d in enumerate(devices)]
        return jax.make_array_from_single_device_arrays(arr2d.shape, core, shards)

    def _put_replicated(arr):
        shards = [jax.device_put(arr, d) for d in devices]
        return jax.make_array_from_single_device_arrays(arr.shape, rep, shards)

    x = np.asarray(inputs["x"]).astype(bf16).reshape(B * NQ, QD)
    puts["x"] = _put_sharded(x)
    ctx = np.asarray(inputs["context"]).astype(bf16).reshape(B * NK, CD)
    puts["context"] = _put_sharded(ctx)
    for nm in ("Wq", "Wk", "Wv", "Wo"):
        puts[nm] = _put_replicated(
            np.ascontiguousarray(np.asarray(inputs[nm]).astype(bf16)))
    puts["bo"] = _put_replicated(
        np.ascontiguousarray(np.asarray(inputs["bo"], dtype=np.float32)))
    puts["out0"] = _put_sharded(np.zeros((B * NQ, QD), bf16))
    _t("enqueue_puts", t0)

    t0 = time.time()
    st = _ensure_compiled()
    _t("build_compile", t0)
    t0 = time.time()
    jax.block_until_ready(list(puts.values()))
    _t("upload_drain", t0)

    args = [puts[nm] for nm in _IN_NAMES] + [puts["out0"]]
    t0 = time.time()
    (out,) = st["compiled"](*args)
    jax.block_until_ready(out)
    exec_s = time.time() - t0
    _t("exec", t0)

    t0 = time.time()
    res = np.empty((B * NQ, QD), np.float32)

    def _fetch(s):
        res[s.index] = np.asarray(s.data, dtype=np.float32)

    shards = list(out.addressable_shards)
    with ThreadPoolExecutor(max_workers=8) as ex:
        list(ex.map(_fetch, shards))
    _t("fetch", t0)
    if dbg:
        print("[kernel timing] " + "  ".join(tlog), flush=True)
    return res.reshape(B, NQ, QD), exec_s


def run(inputs, trace=False):
    """Returns (output, device dispatch+exec seconds)."""
    out, exec_s = _run(inputs)
    return out, exec_s


def kernel(**inputs) -> np.ndarray:
    out, _ = _run(inputs)
    return out


# revision 18
# speedup vs baseline: 20.5603x; 1.2469x over previous
"""Trainium2 Bass kernel for CrossAttention (B=8, Nq=4096, Nk=77, H=16, D=64).

Sharding: data-parallel over batch — one batch element per NeuronCore (8 cores).

End-to-end latency strategy (the dominant cost is host<->device transfer over
the PJRT tunnel plus compile, not kernel exec):
  - All big DRAM I/O is bf16: x upload 64 MB, out download 64 MB.
  - Weights are shipped ONCE (replicated PartitionSpec()) instead of 8 copies.
  - Uploads stream on a worker thread while the main thread builds the Bass
    module and jit-compiles the sharded executable.
  - Output is fetched per-shard in parallel threads, then upcast to f32.

Per-core dataflow (PE matmuls in f32r; bf16 only at the DMA boundary):
  - transpose x chunk on PE (identity matmul)         xT   [1024, CH]
  - qT = Wq^T-free matmul: lhsT=Wq[k,m], rhs=xT[k]    qT   [1024, CH]
  - kT = lhsT=Wk slice, rhs=cT (context transposed)   kT   [1024, 77]
  - v  = lhsT=cT, rhs=Wv (natural layout)             v    [77, 1024] (+ ones col per head)
  - simT_h = lhsT=kT_h [64,77], rhs=qT_h [64,CH]      simT [77, CH]
  - expT_h = exp(scale*simT) on ACT                   expT [77, CH]
  - avT_h  = lhsT=v_aug_h [77,65], rhs=expT           avT  [65, CH] (row 64 = softmax denom)
  - recip + broadcast via tiny matmul, DVE multiply   outT [1024, CH]
  - final = lhsT=outT slice, rhs=Wo  (+ bias, DVE)    out  [CH, 1024] -> DRAM bf16
"""

import os
import sys

for _p in ("/opt/pypackages", "/opt/trn_rl_repo", "/root/.axon_site/_ro/trn_rl_repo",
           "/root/.axon_site/_ro/pypackages"):
    if os.path.isdir(_p) and _p not in sys.path:
        sys.path.append(_p)

import numpy as np

B = 8
NQ = 4096
NK = 77
QD = 1024   # query feature dim
CD = 768    # context feature dim
ID = 1024   # inner dim (= H * D)
H = 16
D = 64
SCALE = D ** -0.5
CH = 512    # seq chunk per pipeline iteration
NCHUNK = NQ // CH
P = 128
NK2 = 78  # NK padded even for fp32r moving/dst

_STATE: dict = {}


def _build():
    import concourse.bass as bass
    import concourse.tile as tile
    from concourse import bacc, mybir
    from concourse.masks import make_identity

    F32 = mybir.dt.float32
    F32R = mybir.dt.float32r
    BF16 = mybir.dt.bfloat16
    AF = mybir.ActivationFunctionType
    ALU = mybir.AluOpType

    nc = bacc.Bacc("TRN2", target_bir_lowering=False, debug=False)

    x_d = nc.dram_tensor("x", [NQ, QD], BF16, kind="ExternalInput").ap()
    ctx_d = nc.dram_tensor("context", [NK, CD], BF16, kind="ExternalInput").ap()
    wq_d = nc.dram_tensor("Wq", [QD, ID], BF16, kind="ExternalInput").ap()
    wk_d = nc.dram_tensor("Wk", [CD, ID], BF16, kind="ExternalInput").ap()
    wv_d = nc.dram_tensor("Wv", [CD, ID], BF16, kind="ExternalInput").ap()
    wo_d = nc.dram_tensor("Wo", [ID, QD], BF16, kind="ExternalInput").ap()
    bo_d = nc.dram_tensor("bo", [QD], F32, kind="ExternalInput").ap()
    out_d = nc.dram_tensor("out", [NQ, QD], BF16, kind="ExternalOutput").ap()

    KQ = QD // P   # 8 k-tiles for x/Wq
    KC = CD // P   # 6 k-tiles for context/Wk/Wv
    KO = ID // P   # 8 k-tiles for Wo

    with tile.TileContext(nc) as tc:
        with (
            tc.tile_pool(name="singles", bufs=1) as singles,
            tc.tile_pool(name="xn_pool", bufs=CH // P + 1) as xn_pool,
            tc.tile_pool(name="wstage", bufs=2) as wstage_pool,
            tc.tile_pool(name="xt_pool", bufs=KQ) as xt_pool,
            tc.tile_pool(name="qt_pool", bufs=KQ) as qt_pool,
            tc.tile_pool(name="ot_pool", bufs=KO) as ot_pool,
            tc.tile_pool(name="expt_pool", bufs=3) as expt_pool,
            tc.tile_pool(name="recip_pool", bufs=2) as recip_pool,
            tc.tile_pool(name="fin_pool", bufs=2) as fin_pool,
            tc.tile_pool(name="ps_small", bufs=4, space="PSUM") as ps_small,
            tc.tile_pool(name="ps_q", bufs=2, space="PSUM") as ps_q,
            tc.tile_pool(name="ps_wo", bufs=2, space="PSUM") as ps_wo,
        ):
            # ---------------- one-time setup ----------------
            ident = singles.tile([P, P], F32, tag="ident")
            make_identity(nc, ident)
            identb = singles.tile([P, P], BF16, tag="identb")
            nc.vector.tensor_copy(identb[:, :], ident[:, :])

            # ones row for broadcasting per-head 1/denom across 64 partitions
            ones_f32 = singles.tile([NK, D], F32, tag="ones_f32")
            nc.gpsimd.memset(ones_f32[:, :], 1.0)
            ones_col = singles.tile([1, D], F32R, tag="ones_col")
            nc.vector.tensor_copy(ones_col[:, :], ones_f32[0:1, :])

            # bias broadcast to all 128 partitions via partition-step-0 DMA
            bias_sb = singles.tile([P, QD], F32, tag="bias")
            bo_bcast = bass.AP(
                tensor=bo_d.tensor, offset=bo_d.offset,
                ap=[[0, P], list(bo_d.ap[0])],
            )
            nc.gpsimd.dma_start(out=bias_sb[:, :], in_=bo_bcast)

            # weights: DMA bf16 staging, then widening copy into fp32r tiles
            wq_sb = [singles.tile([P, ID], F32R, tag=f"wq{k}", name=f"wq{k}") for k in range(KQ)]
            for k in range(KQ):
                stg = wstage_pool.tile([P, ID], BF16, tag="wstage", name="wstage")
                nc.sync.dma_start(out=stg[:, :], in_=wq_d[k * P:(k + 1) * P, :])
                nc.vector.tensor_copy(wq_sb[k][:, :], stg[:, :])
            wk_sb = [singles.tile([P, ID], F32R, tag=f"wk{k}", name=f"wk{k}") for k in range(KC)]
            for k in range(KC):
                stg = wstage_pool.tile([P, ID], BF16, tag="wstage", name="wstage")
                nc.sync.dma_start(out=stg[:, :], in_=wk_d[k * P:(k + 1) * P, :])
                nc.vector.tensor_copy(wk_sb[k][:, :], stg[:, :])
            wv_sb = [singles.tile([P, ID], F32R, tag=f"wv{k}", name=f"wv{k}") for k in range(KC)]
            for k in range(KC):
                stg = wstage_pool.tile([P, ID], BF16, tag="wstage", name="wstage")
                nc.sync.dma_start(out=stg[:, :], in_=wv_d[k * P:(k + 1) * P, :])
                nc.vector.tensor_copy(wv_sb[k][:, :], stg[:, :])
            wo_sb = [singles.tile([P, QD], F32R, tag=f"wo{k}", name=f"wo{k}") for k in range(KO)]
            for k in range(KO):
                stg = wstage_pool.tile([P, QD], BF16, tag="wstage", name="wstage")
                nc.sync.dma_start(out=stg[:, :], in_=wo_d[k * P:(k + 1) * P, :])
                nc.vector.tensor_copy(wo_sb[k][:, :], stg[:, :])

            # context: load natural, transpose to cT tiles [128, 77] x 6
            ctx_sb = singles.tile([NK, CD], BF16, tag="ctx")
            nc.sync.dma_start(out=ctx_sb[:, :], in_=ctx_d[:, :])
            zeros_f32 = singles.tile([P, 1], F32, tag="zeros_f32")
            nc.gpsimd.memset(zeros_f32[:, :], 0.0)
            ct_sb = [singles.tile([P, NK2], F32R, tag=f"ct{k}", name=f"ct{k}") for k in range(KC)]
            for k in range(KC):
                pt = ps_small.tile([P, NK], BF16, tag="ps_attn")
                nc.tensor.transpose(pt[:, :], ctx_sb[:, k * P:(k + 1) * P],
                                    identb[0:NK, 0:NK])
                nc.vector.tensor_copy(ct_sb[k][:, 0:NK], pt[:, :])
                nc.vector.tensor_copy(ct_sb[k][:, NK:NK2], zeros_f32[:, :])

            # kT tiles [128, 77] x 8 (inner dim on partitions)
            kt_sb = [singles.tile([P, NK2], F32R, tag=f"kt{m}", name=f"kt{m}") for m in range(KQ)]
            for m in range(KQ):
                pk = ps_small.tile([P, NK2], F32, tag="ps_attn")
                for k in range(KC):
                    nc.tensor.matmul(
                        pk[:, :], wk_sb[k][:, m * P:(m + 1) * P], ct_sb[k][:, :],
                        start=(k == 0), stop=(k == KC - 1))
                nc.vector.tensor_copy(kt_sb[m][:, :], pk[:, :])

            # v natural [77, 1024] into v_aug [77, 16*65] with ones col per head
            v_aug = singles.tile([NK, H * (D + 1)], F32R, tag="vaug")
            for h in range(H):
                nc.vector.tensor_copy(
                    v_aug[:, h * (D + 1) + D: (h + 1) * (D + 1)], ones_f32[:, 0:1])
            for n in range(2):
                pv = ps_wo.tile([NK, 512], F32, tag="ps_wo")
                for k in range(KC):
                    nc.tensor.matmul(
                        pv[:, :], ct_sb[k][:, 0:NK], wv_sb[k][:, n * 512:(n + 1) * 512],
                        start=(k == 0), stop=(k == KC - 1))
                for hh in range(8):
                    h = n * 8 + hh
                    nc.vector.tensor_copy(
                        v_aug[:, h * (D + 1): h * (D + 1) + D],
                        pv[:, hh * D:(hh + 1) * D])

            # ---------------- main loop over seq chunks ----------------
            for c in range(NCHUNK):
                # load x natural: CH rows of x -> CH//P tiles [128, QD]
                xn = []
                for s in range(CH // P):
                    t = xn_pool.tile([P, QD], BF16, tag="xn", name="xn")
                    nc.sync.dma_start(
                        out=t[:, :],
                        in_=x_d[c * CH + s * P: c * CH + (s + 1) * P, :])
                    xn.append(t)

                # transpose to xT tiles [128, CH] x 8; one wide PSUM evict per tile
                xt = []
                for k in range(KQ):
                    t = xt_pool.tile([P, CH], F32R, tag="xt", name="xt")
                    pt = ps_small.tile([P, CH], BF16, tag="ps_attn")
                    for s in range(CH // P):
                        nc.tensor.transpose(
                            pt[:, s * P:(s + 1) * P], xn[s][:, k * P:(k + 1) * P],
                            identb[:, :])
                    nc.vector.tensor_copy(t[:, :], pt[:, :])
                    xt.append(t)

                # qT tiles [128, CH] x 8
                qt = []
                for m in range(KQ):
                    pq = ps_q.tile([P, CH], F32, tag="ps_q")
                    for k in range(KQ):
                        nc.tensor.matmul(
                            pq[:, :], wq_sb[k][:, m * P:(m + 1) * P], xt[k][:, :],
                            start=(k == 0), stop=(k == KQ - 1))
                    t = qt_pool.tile([P, CH], F32R, tag="qt")
                    nc.vector.tensor_copy(t[:, :], pq[:, :])
                    qt.append(t)

                # attention per head-pair
                ot = [ot_pool.tile([P, CH], F32R, tag="ot", name="ot") for _ in range(KO)]
                for h in range(H):
                    mt = h // 2   # which kT/qT tile
                    lo = (h % 2) * D
                    psim = ps_small.tile([NK, CH], F32, tag="ps_attn")
                    nc.tensor.matmul(
                        psim[:, :],
                        kt_sb[mt][lo:lo + D, 0:NK], qt[mt][lo:lo + D, :],
                        start=True, stop=True)
                    et = expt_pool.tile([NK, CH], F32R, tag="expt")
                    nc.scalar.activation(et[:, :], psim[:, :], AF.Exp,
                                         scale=float(SCALE))
                    pav = ps_small.tile([D + 1, CH], F32, tag="ps_attn")
                    nc.tensor.matmul(
                        pav[:, :],
                        v_aug[:, h * (D + 1): (h + 1) * (D + 1)], et[:, :],
                        start=True, stop=True)
                    rc = recip_pool.tile([1, CH], F32R, tag="recip")
                    with nc.allow_low_precision(reason="fp32r rounding of 1/denom"):
                        nc.vector.reciprocal(rc[:, :], pav[D:D + 1, :])
                    # broadcast 1/denom across 64 partitions via K=1 matmul
                    pb = ps_small.tile([D, CH], F32, tag="ps_attn")
                    nc.tensor.matmul(pb[:, :], ones_col[:, :], rc[:, :],
                                     start=True, stop=True)
                    pb_sb = recip_pool.tile([D, CH], F32, tag="pb_sb", name="pb_sb")
                    nc.vector.tensor_copy(pb_sb[:, :], pb[:, :])
                    nc.vector.tensor_tensor(
                        ot[mt][lo:lo + D, :],
                        pav[0:D, :], pb_sb[:, :], op=ALU.mult)

                # output projection + bias
                for s in range(CH // P):
                    for n in range(QD // 512):
                        po = ps_wo.tile([P, 512], F32, tag="ps_wo")
                        for k in range(KO):
                            nc.tensor.matmul(
                                po[:, :],
                                ot[k][:, s * P:(s + 1) * P],
                                wo_sb[k][:, n * 512:(n + 1) * 512],
                                start=(k == 0), stop=(k == KO - 1))
                        ft = fin_pool.tile([P, 512], BF16, tag="fin")
                        nc.vector.tensor_tensor(
                            ft[:, :], po[:, :], bias_sb[:, n * 512:(n + 1) * 512],
                            op=ALU.add)
                        nc.sync.dma_start(
                            out=out_d[c * CH + s * P: c * CH + (s + 1) * P,
                                      n * 512:(n + 1) * 512],
                            in_=ft[:, :])

    nc.compile()
    return nc


# DRAM input order must match _build's dram_tensor creation order.
_IN_NAMES = ("x", "context", "Wq", "Wk", "Wv", "Wo", "bo")
# Which inputs are sharded over cores (axis 0) vs replicated to all cores.
_SHARDED = {"x", "context"}


def _ensure_compiled():
    """Build the Bass module and AOT-compile the sharded executable once."""
    if "compiled" in _STATE:
        return _STATE

    import jax
    import ml_dtypes
    from jax.sharding import Mesh, PartitionSpec, NamedSharding

    try:
        from jax import shard_map
    except ImportError:
        from jax.experimental.shard_map import shard_map

    from concourse import bass2jax, mybir
    from concourse.bass2jax import _bass_exec_p, install_neuronx_cc_hook

    install_neuronx_cc_hook()

    nc = _build()

    devices = jax.devices()[:B]
    mesh = Mesh(np.asarray(devices), ("core",))
    core = NamedSharding(mesh, PartitionSpec("core"))
    rep = NamedSharding(mesh, PartitionSpec())

    # Bacc implicitly declares a partition_id ExternalInput; it must be the
    # LAST bass_exec operand (generated on-device via PartitionIdOp), and its
    # name must ride last in in_names — exactly what run_bass_via_pjrt does.
    partition_name = nc.partition_id_tensor.name if nc.partition_id_tensor else None
    out_avals = []
    out_names = []
    for alloc in nc.m.functions[0].allocations:
        if not isinstance(alloc, mybir.MemoryLocationSet):
            continue
        if alloc.kind == "ExternalOutput":
            out_names.append(alloc.memorylocations[0].name)
            out_avals.append(jax.core.ShapedArray(
                tuple(alloc.tensor_shape), mybir.dt.np(alloc.dtype)))
    # No output operand: the kernel writes every element of `out`, so PJRT's
    # uninitialized custom-call result allocation is sufficient — skipping the
    # donated-zeros upload entirely (64 MB of wire).
    all_in = list(_IN_NAMES)
    if partition_name is not None:
        all_in.append(partition_name)

    def _body(*args):
        operands = list(args)
        if partition_name is not None:
            operands.append(bass2jax.partition_id_tensor())
        return tuple(_bass_exec_p.bind(
            *operands,
            out_avals=tuple(out_avals),
            in_names=tuple(all_in),
            out_names=tuple(out_names),
            lowering_input_output_aliases=(),
            sim_require_finite=True,
            sim_require_nnan=True,
            nc=nc,
        ))

    in_specs = tuple(
        PartitionSpec("core") if nm in _SHARDED else PartitionSpec()
        for nm in _IN_NAMES
    )
    try:
        smapped = shard_map(_body, mesh=mesh, in_specs=in_specs,
                            out_specs=(PartitionSpec("core"),), check_vma=False)
    except TypeError:
        smapped = shard_map(_body, mesh=mesh, in_specs=in_specs,
                            out_specs=(PartitionSpec("core"),), check_rep=False)
    sharded = jax.jit(smapped, keep_unused=True)

    bf16 = ml_dtypes.bfloat16
    arg_structs = []
    for nm in _IN_NAMES:
        if nm == "x":
            arg_structs.append(jax.ShapeDtypeStruct((B * NQ, QD), bf16, sharding=core))
        elif nm == "context":
            arg_structs.append(jax.ShapeDtypeStruct((B * NK, CD), bf16, sharding=core))
        elif nm == "Wq":
            arg_structs.append(jax.ShapeDtypeStruct((QD, ID), bf16, sharding=rep))
        elif nm in ("Wk", "Wv"):
            arg_structs.append(jax.ShapeDtypeStruct((CD, ID), bf16, sharding=rep))
        elif nm == "Wo":
            arg_structs.append(jax.ShapeDtypeStruct((ID, QD), bf16, sharding=rep))
        elif nm == "bo":
            arg_structs.append(jax.ShapeDtypeStruct((QD,), np.float32, sharding=rep))

    compiled = sharded.lower(*arg_structs).compile()

    _STATE.update(dict(compiled=compiled, mesh=mesh, core=core, rep=rep))
    return _STATE


def _run(inputs):
    """Enqueue async uploads, compile while they stream, execute, fetch."""
    import time
    from concurrent.futures import ThreadPoolExecutor

    import jax
    import ml_dtypes
    from jax.sharding import Mesh, PartitionSpec, NamedSharding

    bf16 = ml_dtypes.bfloat16
    devices = jax.devices()[:B]
    assert len(devices) == B

    mesh = Mesh(np.asarray(devices), ("core",))
    core = NamedSharding(mesh, PartitionSpec("core"))
    rep = NamedSharding(mesh, PartitionSpec())

    dbg = bool(os.environ.get("BASSK_DEBUG"))
    tlog = []

    def _t(label, t0):
        if dbg:
            tlog.append(f"{label}: {time.time() - t0:.2f}s")

    # Upload via raw per-device puts + make_array_from_single_device_arrays.
    # NamedSharding device_puts route through a resharding program that
    # intermittently hits a 1-2 min pathological path on cold tunnels; plain
    # single-device transfers have never shown it. Enqueue before compiling.
    t0 = time.time()
    puts: dict = {}

    def _put_sharded(arr2d):
        n = arr2d.shape[0] // B
        shards = [jax.device_put(arr2d[i * n:(i + 1) * n], d)
                  for i, d in enumerate(devices)]
        return jax.make_array_from_single_device_arrays(arr2d.shape, core, shards)

    def _put_replicated(arr):
        shards = [jax.device_put(arr, d) for d in devices]
        return jax.make_array_from_single_device_arrays(arr.shape, rep, shards)

    x = np.asarray(inputs["x"]).astype(bf16).reshape(B * NQ, QD)
    puts["x"] = _put_sharded(x)
    ctx = np.asarray(inputs["context"]).astype(bf16).reshape(B * NK, CD)
    puts["context"] = _put_sharded(ctx)
    for nm in ("Wq", "Wk", "Wv", "Wo"):
        puts[nm] = _put_replicated(
            np.ascontiguousarray(np.asarray(inputs[nm]).astype(bf16)))
    puts["bo"] = _put_replicated(
        np.ascontiguousarray(np.asarray(inputs["bo"], dtype=np.float32)))
    _t("enqueue_puts", t0)

    t0 = time.time()
    st = _ensure_compiled()
    _t("build_compile", t0)
    t0 = time.time()
    jax.block_until_ready(list(puts.values()))
    _t("upload_drain", t0)

    args = [puts[nm] for nm in _IN_NAMES]
    t0 = time.time()
    (out,) = st["compiled"](*args)
    jax.block_until_ready(out)
    exec_s = time.time() - t0
    _t("exec", t0)

    t0 = time.time()
    res = np.empty((B * NQ, QD), np.float32)

    def _fetch(s):
        res[s.index] = np.asarray(s.data, dtype=np.float32)

    shards = list(out.addressable_shards)
    with ThreadPoolExecutor(max_workers=8) as ex:
        list(ex.map(_fetch, shards))
    _t("fetch", t0)
    if dbg:
        print("[kernel timing] " + "  ".join(tlog), flush=True)
    return res.reshape(B, NQ, QD), exec_s


def run(inputs, trace=False):
    """Returns (output, device dispatch+exec seconds)."""
    out, exec_s = _run(inputs)
    return out, exec_s


def kernel(**inputs) -> np.ndarray:
    out, _ = _run(inputs)
    return out
